# revision 7
# baseline (speedup 1.0000x reference)
"""AdaptiveConv2DMod kernel for 8 TRN2 NeuronCores.

Data-parallel over batch: B=16 -> 2 samples per core, base weights replicated.
Per sample: softmax-mix 4 base kernels, modulate by (1+mod) over input
channels, demodulate per output channel, then 3x3 same-conv.

Conv is computed as 9 shifted matmuls (x2 input-channel chunks) accumulated
in PSUM, bf16 compute / fp32 accumulate.
"""

from contextlib import ExitStack

import numpy as np

import concourse.bass as bass
import concourse.mybir as mybir
import concourse.tile as tile
from concourse import bacc
from concourse.bass_utils import run_bass_kernel_spmd

F32 = mybir.dt.float32
BF16 = mybir.dt.bfloat16

N_CORES = 8
B_LOC = 2          # samples per core
C = 256            # input channels (I)
O = 256            # output channels
H = W = 64
K = 3
NK = 4             # num base kernels
CI = 2             # input channel chunks of 128
CO = 2             # output channel chunks of 128
NT = 8             # row tiles (8 rows x 64 cols = 512 free)
ROWS_PER_NT = H // NT
WP = W + 2         # column-padded width
IKL = C * K * K // CI   # 2304 free elems per (o-part, co) weight tile per... (full i)
# w_nat[n][co]: [128(o), C*K*K=2304] layout (i, kl)
# w_T[b][ci]:   [128(i), K*K*O=2304] layout (kl, o)


def _build_nc(repeat=1):
    nc = bacc.Bacc("TRN2", target_bir_lowering=False, debug=False,
                   num_devices=N_CORES)
    fmap = nc.declare_dram_parameter("fmap", [B_LOC, C, H, W], F32, isOutput=False)
    mod = nc.declare_dram_parameter("mod", [B_LOC, C], F32, isOutput=False)
    kmod = nc.declare_dram_parameter("kernel_mod", [B_LOC, NK], F32, isOutput=False)
    weights = nc.declare_dram_parameter("weights", [NK, O, C, K, K], F32,
                                        isOutput=False)
    out = nc.declare_dram_parameter("out", [B_LOC, O, H, W], F32, isOutput=True)

    with ExitStack() as ctx:
        tc = ctx.enter_context(tile.TileContext(nc))
        pools = _make_pools(ctx, tc)
        for _ in range(repeat):
            _build_body(tc, pools, fmap.ap(), mod.ap(), kmod.ap(),
                        weights.ap(), out.ap())
    nc.compile()
    return nc


def _make_pools(ctx, tc):
    return {
        "const": ctx.enter_context(tc.tile_pool(name="const", bufs=2)),
        "wnat": ctx.enter_context(tc.tile_pool(name="wnat", bufs=NK * CO)),
        "mix": ctx.enter_context(tc.tile_pool(name="mix", bufs=4)),
        "wt": ctx.enter_context(tc.tile_pool(name="wt", bufs=4)),
        "fm": ctx.enter_context(tc.tile_pool(name="fm", bufs=4)),
        "fmraw": ctx.enter_context(tc.tile_pool(name="fmraw", bufs=2)),
        "outp": ctx.enter_context(tc.tile_pool(name="outp", bufs=6)),
        "small": ctx.enter_context(tc.tile_pool(name="small", bufs=10)),
        "psconv": ctx.enter_context(
            tc.tile_pool(name="psconv", bufs=8, space="PSUM")),
    }


def _build_body(tc, pools, fmap, mod, kmod, weights, out):
    nc = tc.nc

    const = pools["const"]
    wnatp = pools["wnat"]
    mixp = pools["mix"]
    wtp = pools["wt"]
    fmp = pools["fm"]
    fmrawp = pools["fmraw"]
    outp = pools["outp"]
    smallp = pools["small"]
    psconv = pools["psconv"]

    # ---- small inputs, broadcast across partitions -------------------------
    # kernel_mod [B,NK] broadcast to all 128 partitions; each partition
    # computes the same softmax redundantly.
    kmod_bc = const.tile([128, B_LOC, NK], F32)
    nc.gpsimd.dma_start(out=kmod_bc[:], in_=kmod[None, :, :].broadcast_to(
        [128, B_LOC, NK]))
    # mod [B, C] -> [128(i), B, CI] (partition = i within chunk)
    modm = const.tile([128, B_LOC, CI], F32)
    nc.gpsimd.dma_start(
        out=modm[:], in_=mod.rearrange("b (c p) -> p b c", p=128))
    # 1 + mod
    nc.vector.tensor_scalar_add(modm[:], modm[:], 1.0)
    # mod broadcast over partitions for the natural-layout weight modulation:
    # m_bc[p, b, i_full] = 1 + mod[b, i_full]
    m_bc = const.tile([128, B_LOC, C], F32)
    nc.gpsimd.dma_start(out=m_bc[:], in_=mod[None, :, :].broadcast_to(
        [128, B_LOC, C]))
    nc.vector.tensor_scalar_add(m_bc[:], m_bc[:], 1.0)

    eps = const.tile([128, 1], F32)
    nc.vector.memset(eps[:], 1e-8)

    # softmax over NK (no max-subtraction; inputs are ~N(0,1))
    esum = const.tile([128, B_LOC], F32)
    attn = const.tile([128, B_LOC, NK], F32)
    nc.scalar.activation(attn[:], kmod_bc[:], mybir.ActivationFunctionType.Exp)
    nc.vector.reduce_sum(esum[:], attn[:], mybir.AxisListType.X)
    nc.vector.reciprocal(esum[:], esum[:])
    for b in range(B_LOC):
        nc.vector.tensor_scalar_mul(attn[:, b, :], attn[:, b, :], esum[:, b:b + 1])

    # ---- load base weights (fp32 -> bf16 cast in DMA) ----------------------
    # w_nat[n][co]: [128(o), (i,kl)=2304] bf16, natural contiguous layout
    w_nat = [[None] * CO for _ in range(NK)]
    for co in range(CO):
        for n in range(NK):
            t = wnatp.tile([128, C, K * K], BF16, tag="wnat")
            nc.gpsimd.dma_start(
                out=t[:], in_=weights[n, co * 128:(co + 1) * 128, :, :, :])
            w_nat[n][co] = t

    # ---- fmap load + column-pad (per sample, per ci chunk) -----------------
    fm_cp = [[None] * CI for _ in range(B_LOC)]

    def load_fmap(b):
        for ci in range(CI):
            raw = fmrawp.tile([128, H, W], BF16, tag="fmraw")
            nc.gpsimd.dma_start(
                out=raw[:], in_=fmap[b, ci * 128:(ci + 1) * 128, :, :])
            t = fmp.tile([128, H, WP], BF16, tag="fmcp")
            # zero pad columns 0 and WP-1
            nc.vector.memset(t[:, :, 0:1], 0.0)
            nc.vector.memset(t[:, :, WP - 1:WP], 0.0)
            nc.vector.tensor_copy(t[:, :, 1:W + 1], raw[:])
            fm_cp[b][ci] = t

    load_fmap(0)
    load_fmap(1)

    # ---- per-sample weight pipeline + conv ---------------------------------
    for b in range(B_LOC):
        # mix + modulate in natural layout, then demod-denominator + transpose
        w_T = [wtp.tile([128, K * K, O], BF16, tag="wt", name=f"wT{b}_{ci}")
               for ci in range(CI)]
        dscale = []
        for co in range(CO):
            t0 = mixp.tile([128, C, K * K], BF16, tag="mixa")
            t1 = mixp.tile([128, C, K * K], BF16, tag="mixb")
            nc.vector.tensor_scalar_mul(t0[:], w_nat[0][co][:], attn[:, b, 0:1])
            nc.vector.tensor_scalar_mul(t1[:], w_nat[1][co][:], attn[:, b, 1:2])
            nc.vector.tensor_add(t0[:], t0[:], t1[:])
            nc.vector.tensor_scalar_mul(t1[:], w_nat[2][co][:], attn[:, b, 2:3])
            nc.vector.tensor_add(t0[:], t0[:], t1[:])
            nc.vector.tensor_scalar_mul(t1[:], w_nat[3][co][:], attn[:, b, 3:4])
            nc.vector.tensor_add(t0[:], t0[:], t1[:])
            # modulate: w *= (1 + mod[i]), i varies along free dim.
            # Output written in (kl, i) order so each tap slice is contiguous
            # for the xbar transpose below.
            wmod = mixp.tile([128, K * K, C], BF16, tag="wmod")
            nc.vector.tensor_mul(
                wmod.rearrange("p kl c -> p c kl"), t0[:],
                m_bc[:, b, :, None].broadcast_to([128, C, K * K]))
            # demod denominator: sum over free dims of wmod^2 (per o-partition)
            sqscratch = mixp.tile([128, K * K, C], BF16, tag="sqs")
            den = smallp.tile([128, 1], F32, tag="den")
            nc.scalar.activation(
                sqscratch[:], wmod[:],
                mybir.ActivationFunctionType.Square, accum_out=den[:])
            # dscale = 1/sqrt(den + eps)
            ds = smallp.tile([128, 1], F32, tag="dsc")
            nc.scalar.activation(ds[:], den[:],
                                 mybir.ActivationFunctionType.Sqrt, bias=eps[:])
            nc.vector.reciprocal(ds[:], ds[:])
            dscale.append(ds)
            # transpose [o, i] -> [i, o] per (ci, tap) via DMA xbar
            for ci in range(CI):
                for kl in range(K * K):
                    nc.sync.dma_start(
                        out=w_T[ci][:, kl, co * 128:co * 128 + 128],
                        in_=wmod[:, kl, ci * 128:(ci + 1) * 128],
                        transpose=True)

        # ---- conv: out[o, y, x] += sum_{ci,ky,kx} w.T @ fmap_shifted -------
        for co in range(CO):
            ps = [psconv.tile([128, ROWS_PER_NT * W], F32, tag="ps",
                              name=f"ps{b}_{co}_{nt}")
                  for nt in range(NT)]
            first = True
            for ci in range(CI):
                for ky in range(K):
                    for kx in range(K):
                        kl = ky * K + kx
                        lhsT = w_T[ci][:, kl, co * 128:co * 128 + 128]
                        for nt in range(NT):
                            y0 = nt * ROWS_PER_NT
                            r0 = y0 + ky - 1          # first input row
                            ny = ROWS_PER_NT
                            psoff = 0
                            if r0 < 0:                # clamp top (ky=0, nt=0)
                                r0, ny, psoff = 0, ROWS_PER_NT - 1, W
                            if r0 + ny > H:           # clamp bottom
                                ny = H - r0
                            rhs = fm_cp[b][ci][:, r0:r0 + ny, kx:kx + W]
                            is_last = (ci == CI - 1 and kl == K * K - 1)
                            nc.tensor.matmul(
                                ps[nt][:, psoff:psoff + ny * W],
                                lhsT, rhs,
                                start=first, stop=is_last)
                        first = False
            # evacuate with demod scale, DMA out
            for nt in range(NT):
                ot = outp.tile([128, ROWS_PER_NT * W], F32, tag="ot")
                nc.vector.tensor_scalar_mul(ot[:], ps[nt][:], dscale[co][:])
                nc.sync.dma_start(
                    out=out[b, co * 128:(co + 1) * 128,
                            nt * ROWS_PER_NT:(nt + 1) * ROWS_PER_NT, :],
                    in_=ot[:])


_NC_CACHE = {}


def _get_nc(repeat=1):
    if repeat not in _NC_CACHE:
        _NC_CACHE[repeat] = _build_nc(repeat)
    return _NC_CACHE[repeat]


def _make_in_maps(fmap, mod, kernel_mod, weights):
    in_maps = []
    for c in range(N_CORES):
        s = slice(c * B_LOC, (c + 1) * B_LOC)
        in_maps.append({
            "fmap": np.ascontiguousarray(fmap[s]),
            "mod": np.ascontiguousarray(mod[s]),
            "kernel_mod": np.ascontiguousarray(kernel_mod[s]),
            "weights": weights,
        })
    return in_maps


def kernel(fmap, mod, kernel_mod, weights, _trace=False):
    fmap = np.asarray(fmap, dtype=np.float32)
    mod = np.asarray(mod, dtype=np.float32)
    kernel_mod = np.asarray(kernel_mod, dtype=np.float32)
    weights = np.ascontiguousarray(np.asarray(weights, dtype=np.float32))

    nc = _get_nc()
    in_maps = _make_in_maps(fmap, mod, kernel_mod, weights)
    res = run_bass_kernel_spmd(nc, in_maps, list(range(N_CORES)), trace=_trace)
    outs = np.concatenate([res.results[c]["out"] for c in range(N_CORES)], axis=0)
    if _trace:
        kernel.last_results = res
    return outs


# revision 8
# speedup vs baseline: 41.9746x; 41.9746x over previous
"""AdaptiveConv2DMod kernel for 8 TRN2 NeuronCores.

Data-parallel over batch: B=16 -> 2 samples per core, base weights replicated.
Per sample: softmax-mix 4 base kernels, modulate by (1+mod) over input
channels, demodulate per output channel, then 3x3 same-conv.

Conv is computed as 9 shifted matmuls (x2 input-channel chunks) accumulated
in PSUM, bf16 compute / fp32 accumulate.
"""

from contextlib import ExitStack

import numpy as np

import concourse.bass as bass
import concourse.mybir as mybir
import concourse.tile as tile
from concourse import bacc
from concourse.bass_utils import run_bass_kernel_spmd

F32 = mybir.dt.float32
BF16 = mybir.dt.bfloat16

N_CORES = 8
B_LOC = 2          # samples per core
C = 256            # input channels (I)
O = 256            # output channels
H = W = 64
K = 3
NK = 4             # num base kernels
CI = 2             # input channel chunks of 128
CO = 2             # output channel chunks of 128
NT = 8             # row tiles (8 rows x 64 cols = 512 free)
ROWS_PER_NT = H // NT
WP = W + 2         # column-padded width
IKL = C * K * K // CI   # 2304 free elems per (o-part, co) weight tile per... (full i)
# w_nat[n][co]: [128(o), C*K*K=2304] layout (i, kl)
# w_T[b][ci]:   [128(i), K*K*O=2304] layout (kl, o)


def _build_nc(repeat=1, loop_n=0):
    nc = bacc.Bacc("TRN2", target_bir_lowering=False, debug=False,
                   num_devices=N_CORES)
    fmap = nc.declare_dram_parameter("fmap", [B_LOC, C, H, W], F32, isOutput=False)
    mod = nc.declare_dram_parameter("mod", [B_LOC, C], F32, isOutput=False)
    kmod = nc.declare_dram_parameter("kernel_mod", [B_LOC, NK], F32, isOutput=False)
    weights = nc.declare_dram_parameter("weights", [NK, O, C, K, K], F32,
                                        isOutput=False)
    out = nc.declare_dram_parameter("out", [B_LOC, O, H, W], F32, isOutput=True)

    with ExitStack() as ctx:
        tc = ctx.enter_context(tile.TileContext(nc))
        pools = _make_pools(ctx, tc)
        if loop_n:
            with tc.For_i(0, loop_n, 1):
                _build_body(tc, pools, fmap.ap(), mod.ap(), kmod.ap(),
                            weights.ap(), out.ap())
        else:
            for _ in range(repeat):
                _build_body(tc, pools, fmap.ap(), mod.ap(), kmod.ap(),
                            weights.ap(), out.ap())
    nc.compile()
    return nc


def _make_pools(ctx, tc):
    return {
        "const": ctx.enter_context(tc.tile_pool(name="const", bufs=2)),
        "wnat": ctx.enter_context(tc.tile_pool(name="wnat", bufs=NK * CO)),
        "mix": ctx.enter_context(tc.tile_pool(name="mix", bufs=4)),
        "wt": ctx.enter_context(tc.tile_pool(name="wt", bufs=4)),
        "fm": ctx.enter_context(tc.tile_pool(name="fm", bufs=4)),
        "fmraw": ctx.enter_context(tc.tile_pool(name="fmraw", bufs=2)),
        "outp": ctx.enter_context(tc.tile_pool(name="outp", bufs=6)),
        "small": ctx.enter_context(tc.tile_pool(name="small", bufs=10)),
        "psconv": ctx.enter_context(
            tc.tile_pool(name="psconv", bufs=8, space="PSUM")),
    }


def _build_body(tc, pools, fmap, mod, kmod, weights, out):
    nc = tc.nc

    const = pools["const"]
    wnatp = pools["wnat"]
    mixp = pools["mix"]
    wtp = pools["wt"]
    fmp = pools["fm"]
    fmrawp = pools["fmraw"]
    outp = pools["outp"]
    smallp = pools["small"]
    psconv = pools["psconv"]

    # ---- small inputs, broadcast across partitions -------------------------
    # kernel_mod [B,NK] broadcast to all 128 partitions; each partition
    # computes the same softmax redundantly.
    kmod_bc = const.tile([128, B_LOC, NK], F32)
    nc.gpsimd.dma_start(out=kmod_bc[:], in_=kmod[None, :, :].broadcast_to(
        [128, B_LOC, NK]))
    # mod [B, C] -> [128(i), B, CI] (partition = i within chunk)
    modm = const.tile([128, B_LOC, CI], F32)
    nc.gpsimd.dma_start(
        out=modm[:], in_=mod.rearrange("b (c p) -> p b c", p=128))
    # 1 + mod
    nc.vector.tensor_scalar_add(modm[:], modm[:], 1.0)
    # mod broadcast over partitions for the natural-layout weight modulation:
    # m_bc[p, b, i_full] = 1 + mod[b, i_full]
    m_bc = const.tile([128, B_LOC, C], F32)
    nc.gpsimd.dma_start(out=m_bc[:], in_=mod[None, :, :].broadcast_to(
        [128, B_LOC, C]))
    nc.vector.tensor_scalar_add(m_bc[:], m_bc[:], 1.0)

    eps = const.tile([128, 1], F32)
    nc.vector.memset(eps[:], 1e-8)

    # softmax over NK (no max-subtraction; inputs are ~N(0,1))
    esum = const.tile([128, B_LOC], F32)
    attn = const.tile([128, B_LOC, NK], F32)
    nc.scalar.activation(attn[:], kmod_bc[:], mybir.ActivationFunctionType.Exp)
    nc.vector.reduce_sum(esum[:], attn[:], mybir.AxisListType.X)
    nc.vector.reciprocal(esum[:], esum[:])
    for b in range(B_LOC):
        nc.vector.tensor_scalar_mul(attn[:, b, :], attn[:, b, :], esum[:, b:b + 1])

    # ---- load base weights (fp32 -> bf16 cast in DMA) ----------------------
    # w_nat[n][co]: [128(o), (i,kl)=2304] bf16, natural contiguous layout
    w_nat = [[None] * CO for _ in range(NK)]
    for co in range(CO):
        for n in range(NK):
            t = wnatp.tile([128, C, K * K], BF16, tag="wnat")
            nc.gpsimd.dma_start(
                out=t[:], in_=weights[n, co * 128:(co + 1) * 128, :, :, :])
            w_nat[n][co] = t

    # ---- fmap load + column-pad (per sample, per ci chunk) -----------------
    fm_cp = [[None] * CI for _ in range(B_LOC)]

    def load_fmap(b):
        for ci in range(CI):
            raw = fmrawp.tile([128, H, W], BF16, tag="fmraw")
            nc.gpsimd.dma_start(
                out=raw[:], in_=fmap[b, ci * 128:(ci + 1) * 128, :, :])
            t = fmp.tile([128, H, WP], BF16, tag="fmcp")
            # zero pad columns 0 and WP-1
            nc.vector.memset(t[:, :, 0:1], 0.0)
            nc.vector.memset(t[:, :, WP - 1:WP], 0.0)
            nc.vector.tensor_copy(t[:, :, 1:W + 1], raw[:])
            fm_cp[b][ci] = t

    load_fmap(0)
    load_fmap(1)

    # ---- per-sample weight pipeline + conv ---------------------------------
    for b in range(B_LOC):
        # mix + modulate in natural layout, then demod-denominator + transpose
        w_T = [wtp.tile([128, K * K, O], BF16, tag="wt", name=f"wT{b}_{ci}")
               for ci in range(CI)]
        dscale = []
        for co in range(CO):
            t0 = mixp.tile([128, C, K * K], BF16, tag="mixa")
            t1 = mixp.tile([128, C, K * K], BF16, tag="mixb")
            nc.vector.tensor_scalar_mul(t0[:], w_nat[0][co][:], attn[:, b, 0:1])
            nc.vector.tensor_scalar_mul(t1[:], w_nat[1][co][:], attn[:, b, 1:2])
            nc.vector.tensor_add(t0[:], t0[:], t1[:])
            nc.vector.tensor_scalar_mul(t1[:], w_nat[2][co][:], attn[:, b, 2:3])
            nc.vector.tensor_add(t0[:], t0[:], t1[:])
            nc.vector.tensor_scalar_mul(t1[:], w_nat[3][co][:], attn[:, b, 3:4])
            nc.vector.tensor_add(t0[:], t0[:], t1[:])
            # modulate: w *= (1 + mod[i]), i varies along free dim.
            # Output written in (kl, i) order so each tap slice is contiguous
            # for the xbar transpose below.
            wmod = mixp.tile([128, K * K, C], BF16, tag="wmod")
            nc.vector.tensor_mul(
                wmod.rearrange("p kl c -> p c kl"), t0[:],
                m_bc[:, b, :, None].broadcast_to([128, C, K * K]))
            # demod denominator: sum over free dims of wmod^2 (per o-partition)
            sqscratch = mixp.tile([128, K * K, C], BF16, tag="sqs")
            den = smallp.tile([128, 1], F32, tag="den")
            nc.scalar.activation(
                sqscratch[:], wmod[:],
                mybir.ActivationFunctionType.Square, accum_out=den[:])
            # dscale = 1/sqrt(den + eps)
            ds = smallp.tile([128, 1], F32, tag="dsc")
            nc.scalar.activation(ds[:], den[:],
                                 mybir.ActivationFunctionType.Sqrt, bias=eps[:])
            nc.vector.reciprocal(ds[:], ds[:])
            dscale.append(ds)
            # transpose [o, i] -> [i, o] per (ci, tap) via DMA xbar
            for ci in range(CI):
                for kl in range(K * K):
                    nc.sync.dma_start(
                        out=w_T[ci][:, kl, co * 128:co * 128 + 128],
                        in_=wmod[:, kl, ci * 128:(ci + 1) * 128],
                        transpose=True)

        # ---- conv: out[o, y, x] += sum_{ci,ky,kx} w.T @ fmap_shifted -------
        for co in range(CO):
            ps = [psconv.tile([128, ROWS_PER_NT * W], F32, tag="ps",
                              name=f"ps{b}_{co}_{nt}")
                  for nt in range(NT)]
            first = True
            for ci in range(CI):
                for ky in range(K):
                    for kx in range(K):
                        kl = ky * K + kx
                        lhsT = w_T[ci][:, kl, co * 128:co * 128 + 128]
                        for nt in range(NT):
                            y0 = nt * ROWS_PER_NT
                            r0 = y0 + ky - 1          # first input row
                            ny = ROWS_PER_NT
                            psoff = 0
                            if r0 < 0:                # clamp top (ky=0, nt=0)
                                r0, ny, psoff = 0, ROWS_PER_NT - 1, W
                            if r0 + ny > H:           # clamp bottom
                                ny = H - r0
                            rhs = fm_cp[b][ci][:, r0:r0 + ny, kx:kx + W]
                            is_last = (ci == CI - 1 and kl == K * K - 1)
                            nc.tensor.matmul(
                                ps[nt][:, psoff:psoff + ny * W],
                                lhsT, rhs,
                                start=first, stop=is_last)
                        first = False
            # evacuate with demod scale, DMA out
            for nt in range(NT):
                ot = outp.tile([128, ROWS_PER_NT * W], F32, tag="ot")
                nc.vector.tensor_scalar_mul(ot[:], ps[nt][:], dscale[co][:])
                nc.sync.dma_start(
                    out=out[b, co * 128:(co + 1) * 128,
                            nt * ROWS_PER_NT:(nt + 1) * ROWS_PER_NT, :],
                    in_=ot[:])


_NC_CACHE = {}


def _get_nc(repeat=1, loop_n=0):
    key = (repeat, loop_n)
    if key not in _NC_CACHE:
        _NC_CACHE[key] = _build_nc(repeat, loop_n)
    return _NC_CACHE[key]


def _make_in_maps(fmap, mod, kernel_mod, weights):
    in_maps = []
    for c in range(N_CORES):
        s = slice(c * B_LOC, (c + 1) * B_LOC)
        in_maps.append({
            "fmap": np.ascontiguousarray(fmap[s]),
            "mod": np.ascontiguousarray(mod[s]),
            "kernel_mod": np.ascontiguousarray(kernel_mod[s]),
            "weights": weights,
        })
    return in_maps


def kernel(fmap, mod, kernel_mod, weights, _trace=False):
    fmap = np.asarray(fmap, dtype=np.float32)
    mod = np.asarray(mod, dtype=np.float32)
    kernel_mod = np.asarray(kernel_mod, dtype=np.float32)
    weights = np.ascontiguousarray(np.asarray(weights, dtype=np.float32))

    nc = _get_nc()
    in_maps = _make_in_maps(fmap, mod, kernel_mod, weights)
    res = run_bass_kernel_spmd(nc, in_maps, list(range(N_CORES)), trace=_trace)
    outs = np.concatenate([res.results[c]["out"] for c in range(N_CORES)], axis=0)
    if _trace:
        kernel.last_results = res
    return outs
